# revision 21
# baseline (speedup 1.0000x reference)
"""BitLinear (bit-decoded weights + STE quant) Trainium2 kernel, v4.

y = x @ W^T + b, where
  W = decode_bits(bweight, wsign) * scale,  b = decode_bits(bbias, bsign)
      * biasscale; STE forward == identity on the already-quantized decode.

Decode: n = sum_k bits[..., k] * 2^(7-k) (exact integers 0..255),
        W = n * sign(wsign) * (scale/255).

Device strategy per core (2 token-groups x 4 out-feature-groups grid):
  - the bit-plane summation runs ON THE PE as matmuls against a small
    constant decode matrix D: contraction dim = (32 input rows x 4 bit
    planes); D[(i,k), i'] = delta_{i,i'} * 2^(3-k) * (16 for the hi
    nibble, 1 for lo). Two accumulating fp8 matmuls (hi+lo) per 32-row
    block produce n = sum_k 2^(7-k) b_k exactly in fp32 PSUM; 4 blocks
    run CONCURRENTLY in the 128x128 array via tile_position column
    tiling, so decoding all 2048x512 weights costs ~7 us of PE time.
    Raw {0,1} bit planes stream over plain HWDGE DMA (fp8, 8 MiB).
  - ACT evicts n to fp16 (exact, <= 255); ACT computes sign(wsign) in
    place; one DVE multiply forms W_int = n * sign.
  - main matmul: psum[o=128, t=512] += W_int^T[i,o-blk] @ x^T[i,t-chunk]
    in fp16, fp32 PSUM accumulation; t-group-major with triple-buffered
    x streaming, 6 PSUM banks (2 reserved for decode).
  - eviction on ACT: y^T = Identity(psum * (scale/255) + bias_o), bf16
    output (host upcasts to f32; 2^-9 rounding ~10x under tolerance).
  - weights/bias/scalars are double-buffered across repeat iterations so
    iteration k+1's decode overlaps iteration k's matmuls: the in-NEFF
    repeat loop measures steady-state (pipelined) throughput.

Distribution over 8 NeuronCores: no collectives - each core writes its
own y^T shard, host reassembles.

Host-side work is layout/precision only: transposes, shard slicing, and
dtype conversion (bits {0.,1.} -> fp8 exact, x -> fp16, wsign -> bf16).
All of the module's arithmetic (bit-plane weighting and summation, sign,
scaling, matmul, bias) runs on the device. The decode matrix D is an
algorithm constant (like a transpose identity), not input data.
"""

import numpy as np

import concourse.mybir as mybir
import concourse.tile as tile
from concourse import bacc
from concourse import bass_utils

# ---- problem constants (hardcoded per contract) ----
B, S, IN, OUT, NB = 4, 2048, 2048, 2048, 8
T = B * S                      # 8192 tokens
P = 128                        # partitions
P_T, P_O = 2, 4                # token-parallel x out-feature-parallel grid
N_CORES = P_T * P_O
T_SH = T // P_T                # 4096
O_SH = OUT // P_O              # 512
KB = IN // P                   # 16 contraction blocks
OB = O_SH // P                 # 4 out blocks
TGW = 512                      # t-group width
NT = T_SH // TGW               # 8 t-groups per core

F32 = mybir.dt.float32
FP16 = mybir.dt.float16
BF16 = mybir.dt.bfloat16
FP8 = mybir.dt.float8e4
AL = mybir.AluOpType
IDENT = mybir.ActivationFunctionType.Identity

_CACHE = {}


def _pairs(ap):
    """Split the last (fast) axis of a [..., 2n] AP into even/odd views."""
    v = ap.rearrange("p (c two) -> p c two", two=2)
    return v[:, :, 0], v[:, :, 1]


def _decode_matrix():
    """D [128, 64] fp8: cols 0-31 hi-nibble decode, 32-63 lo-nibble.
    Row p = (i_sub, k) with i_sub = p//4, k = p%4 (MSB first)."""
    d = np.zeros((P, 64), dtype=np.float32)
    for p in range(P):
        i_sub, k = p // 4, p % 4
        d[p, i_sub] = 16.0 * (2.0 ** (3 - k))      # hi: 128,64,32,16
        d[p, 32 + i_sub] = 2.0 ** (3 - k)          # lo: 8,4,2,1
    return d


def _build_nc(repeats=1):
    nc = bacc.Bacc("TRN2", target_bir_lowering=False, debug=False,
                   num_devices=N_CORES)

    xgd = nc.dram_tensor("xg", [NT * P, KB * TGW], FP16,
                         kind="ExternalInput").ap()
    # raw bit planes, chunk order (kb, j, hi|lo), each chunk [128, 512]
    bits = nc.dram_tensor("bits", [P, KB * 4 * 2 * O_SH], FP8,
                          kind="ExternalInput").ap()
    wsd = nc.dram_tensor("ws", [P, KB * O_SH], BF16,
                         kind="ExternalInput").ap()
    dmat = nc.dram_tensor("dmat", [P, 64], FP8, kind="ExternalInput").ap()
    bb = nc.dram_tensor("bb", [P, OB * NB], F32, kind="ExternalInput").ap()
    bs = nc.dram_tensor("bs", [P, OB], F32, kind="ExternalInput").ap()
    scl = nc.dram_tensor("scl", [P, 1], F32, kind="ExternalInput").ap()
    bscl = nc.dram_tensor("bscl", [P, 1], F32, kind="ExternalInput").ap()
    y = nc.dram_tensor("y", [O_SH, T_SH], BF16, kind="ExternalOutput").ap()

    with tile.TileContext(nc) as tc:
      with tc.tile_pool(name="w", bufs=1) as wpool, \
           tc.tile_pool(name="xs", bufs=1) as xpool, \
           tc.tile_pool(name="dec", bufs=1) as dec, \
           tc.tile_pool(name="yb", bufs=1) as ypool, \
           tc.tile_pool(name="psum", bufs=1, space="PSUM") as psum_pool:

        dm = None
        if repeats:
            dm = dec.tile([P, 64], FP8, name="dmat_sb")
            nc.sync.dma_start(out=dm, in_=dmat)

        for rep in range(repeats):
            par = rep % 2   # parity suffix: double-buffer rep-crossing state

            # ---- scalars ----
            scl_sb = dec.tile([P, 1], F32, name=f"scl_sb{par}")
            nc.sync.dma_start(out=scl_sb, in_=scl)
            bscl_sb = dec.tile([P, 1], F32, name=f"bscl_sb{par}")
            nc.sync.dma_start(out=bscl_sb, in_=bscl)
            s255 = dec.tile([P, 1], F32, name=f"s255_{par}")
            nc.vector.tensor_scalar_mul(s255, scl_sb, 1.0 / 255.0)
            bs255 = dec.tile([P, 1], F32, name=f"bs255_{par}")
            nc.vector.tensor_scalar_mul(bs255, bscl_sb, 1.0 / 255.0)

            # ---- bias decode: bias_col [128, OB] (o on partitions) ----
            bb_sb = dec.tile([P, OB * NB], F32, name=f"bb_sb{par}")
            nc.sync.dma_start(out=bb_sb, in_=bb)
            bs_sb = dec.tile([P, OB], F32, name=f"bs_sb{par}")
            nc.sync.dma_start(out=bs_sb, in_=bs)
            e, o = _pairs(bb_sb)
            bl1 = dec.tile([P, OB * 4], F32, name=f"bl1_{par}")
            nc.vector.scalar_tensor_tensor(out=bl1, in0=e, scalar=2.0,
                                           in1=o, op0=AL.mult, op1=AL.add)
            e, o = _pairs(bl1)
            bl2 = dec.tile([P, OB * 2], F32, name=f"bl2_{par}")
            nc.vector.scalar_tensor_tensor(out=bl2, in0=e, scalar=4.0,
                                           in1=o, op0=AL.mult, op1=AL.add)
            e, o = _pairs(bl2)
            bl3 = dec.tile([P, OB], F32, name=f"bl3_{par}")
            nc.vector.scalar_tensor_tensor(out=bl3, in0=e, scalar=16.0,
                                           in1=o, op0=AL.mult, op1=AL.add)
            bsg = dec.tile([P, OB], F32, name=f"bsg{par}")
            nc.scalar.sign(bsg, bs_sb)
            bias_col = dec.tile([P, OB], F32, name=f"bias_col{par}")
            nc.vector.scalar_tensor_tensor(out=bias_col, in0=bl3,
                                           scalar=bs255, in1=bsg,
                                           op0=AL.mult, op1=AL.mult)

            # ---- weight sign (resident bf16, sign computed in place) ----
            wsg = dec.tile([P, KB * O_SH], BF16, name=f"wsg{par}")
            nc.sync.dma_start(out=wsg, in_=wsd)
            nc.scalar.sign(wsg, wsg)

            # ---- weight decode on the PE: n = D_hi.T@bits_hi + D_lo.T@
            #      bits_lo per 32-row block, 4 blocks col-tiled ----
            W = wpool.tile([P, KB * O_SH], FP16, name=f"W{par}")
            W3 = W.rearrange("p (kb o) -> p kb o", kb=KB)
            CHK = 2 * O_SH                       # hi+lo chunk pair cols
            for kb in range(KB):
                bt = dec.tile([P, 4 * CHK], FP8, tag="bt", bufs=4)
                nc.sync.dma_start(
                    out=bt,
                    in_=bits[:, kb * 4 * CHK:(kb + 1) * 4 * CHK])
                bt3 = bt.rearrange("p (j h o) -> p j h o", j=4, h=2)
                ps = psum_pool.tile([P, O_SH], F32, tag="dec", bufs=2)
                for j in range(4):
                    nc.tensor.matmul(
                        ps[j * 32:(j + 1) * 32, :], dm[:, 0:32],
                        bt3[:, j, 0], start=True, stop=False,
                        tile_position=(0, j * 32), skip_group_check=True)
                    nc.tensor.matmul(
                        ps[j * 32:(j + 1) * 32, :], dm[:, 32:64],
                        bt3[:, j, 1], start=False, stop=True,
                        tile_position=(0, j * 32), skip_group_check=True)
                nc.scalar.activation(out=W3[:, kb], in_=ps, func=IDENT)
            # W_int = n * sign (exact fp16 integers, |n| <= 255)
            nc.vector.tensor_tensor(out=W, in0=W, in1=wsg, op=AL.mult)

            # ---- main matmul: t-group-major, x triple-buffered ----
            def load_xg(g):
                xt = xpool.tile([P, KB * TGW], FP16, tag="xg",
                                name=f"xg{g}_{rep}", bufs=3)
                nc.sync.dma_start(out=xt, in_=xgd[g * P:(g + 1) * P, :])
                return xt

            xtiles = {g: load_xg(g) for g in range(min(3, NT))}
            for g in range(NT):
                xg3 = xtiles[g].rearrange("p (kb t) -> p kb t", kb=KB)
                ybuf = ypool.tile([P, OB * TGW], BF16, tag="yb", bufs=2)
                yb3 = ybuf.rearrange("p (ob t) -> p ob t", ob=OB)
                for ob in range(OB):
                    ps = psum_pool.tile([P, TGW], F32, tag="mm", bufs=6)
                    for kb in range(KB):
                        nc.tensor.matmul(
                            ps,
                            W3[:, kb, ob * P:(ob + 1) * P],
                            xg3[:, kb],
                            start=(kb == 0),
                            stop=(kb == KB - 1),
                        )
                    # y^T tile = psum * (scale/255) + bias_o   (ACT)
                    nc.scalar.activation(
                        out=yb3[:, ob], in_=ps, func=IDENT,
                        bias=bias_col[:, ob:ob + 1], scale=s255)
                # y on the ACT HWDGE ring: it waits on evictions, and on
                # the SP ring it would head-of-line block the x/bits
                # prefetch stream
                nc.scalar.dma_start(
                    out=y.rearrange("(ob p) t -> p ob t", p=P)[
                        :, :, g * TGW:(g + 1) * TGW],
                    in_=yb3,
                )
                if g + 3 < NT:
                    xtiles[g + 3] = load_xg(g + 3)

    nc.compile()
    return nc


def _shard_inputs(x, bweight, wsign, scale, bbias, bsign, biasscale):
    fp8_np = mybir.dt.np(FP8)
    bf16_np = mybir.dt.np(BF16)

    x2 = np.asarray(x, dtype=np.float32).reshape(T, IN)
    bwf = np.asarray(bweight, dtype=np.float32)
    wsf = np.asarray(wsign, dtype=np.float32)
    bbias = np.asarray(bbias, dtype=np.float32)
    bsign = np.asarray(bsign, dtype=np.float32)

    scl_rep = np.full((P, 1), np.asarray(scale).reshape(-1)[0],
                      dtype=np.float32)
    bscl_rep = np.full((P, 1), np.asarray(biasscale).reshape(-1)[0],
                       dtype=np.float32)
    dmat_np = _decode_matrix().astype(fp8_np)

    o_maps = []
    for o_grp in range(P_O):
        osl = slice(o_grp * O_SH, (o_grp + 1) * O_SH)
        bw_sh = bwf[osl]                              # [O_SH, IN, NB]
        # bits chunks [128 = (32 i_sub x 4 k), O_SH], order (kb, j, hi|lo)
        bits_chunks = []
        for kb in range(KB):
            for j in range(4):
                i0 = kb * P + j * 32
                sub = bw_sh[:, i0:i0 + 32, :]         # [O_SH, 32, 8]
                for half in (0, 4):
                    bits_chunks.append(np.ascontiguousarray(
                        sub[:, :, half:half + 4].transpose(1, 2, 0)
                        .reshape(P, O_SH).astype(fp8_np)))
        # ws: [p, kb*O_SH + o] = sign weight for (i = kb*128+p, o)
        ws_np = np.ascontiguousarray(
            wsf[osl].T.reshape(KB, P, O_SH).transpose(1, 0, 2)
            .reshape(P, KB * O_SH)).astype(bf16_np)
        o_maps.append({
            "bits": np.concatenate(bits_chunks, axis=1),
            "ws": ws_np,
            "dmat": dmat_np,
            "bb": np.ascontiguousarray(
                bbias[osl].reshape(OB, P, NB).transpose(1, 0, 2)
                .reshape(P, OB * NB)),
            "bs": np.ascontiguousarray(bsign[osl].reshape(OB, P).T),
            "scl": scl_rep,
            "bscl": bscl_rep,
        })

    in_maps = [None] * N_CORES
    for t_grp in range(P_T):
        tsl = slice(t_grp * T_SH, (t_grp + 1) * T_SH)
        xs = x2[tsl]                                  # [T_SH, IN]
        xg_np = np.ascontiguousarray(
            xs.reshape(NT, TGW, KB, P).transpose(0, 3, 2, 1)
            .reshape(NT * P, KB * TGW).astype(np.float16))
        for o_grp in range(P_O):
            c = t_grp * P_O + o_grp
            in_maps[c] = dict(o_maps[o_grp], xg=xg_np)
    return in_maps


def kernel(x, bweight, wsign, scale, bbias, bsign, biasscale):
    if "nc" not in _CACHE:
        _CACHE["nc"] = _build_nc()
    nc = _CACHE["nc"]
    in_maps = _shard_inputs(x, bweight, wsign, scale, bbias, bsign, biasscale)
    res = bass_utils.run_bass_kernel_spmd(
        nc, in_maps, core_ids=list(range(N_CORES)))
    Y = np.empty((T, OUT), dtype=np.float32)
    for c in range(N_CORES):
        t_grp, o_grp = c // P_O, c % P_O
        Y[t_grp * T_SH:(t_grp + 1) * T_SH,
          o_grp * O_SH:(o_grp + 1) * O_SH] = \
            res.results[c]["y"].T.astype(np.float32)
    return Y.reshape(B, S, OUT)


# revision 26
# speedup vs baseline: 4.1478x; 4.1478x over previous
"""BitLinear (bit-decoded weights + STE quant) Trainium2 kernel, v4.

y = x @ W^T + b, where
  W = decode_bits(bweight, wsign) * scale,  b = decode_bits(bbias, bsign)
      * biasscale; STE forward == identity on the already-quantized decode.

Decode: n = sum_k bits[..., k] * 2^(7-k) (exact integers 0..255),
        W = n * sign(wsign) * (scale/255).

Device strategy per core (2 token-groups x 4 out-feature-groups grid):
  - the bit-plane summation runs ON THE PE as matmuls against a small
    constant decode matrix D: contraction dim = (32 input rows x 4 bit
    planes); D[(i,k), i'] = delta_{i,i'} * 2^(3-k) * (16 for the hi
    nibble, 1 for lo). Two accumulating fp8 matmuls (hi+lo) per 32-row
    block produce n = sum_k 2^(7-k) b_k exactly in fp32 PSUM; 4 blocks
    run CONCURRENTLY in the 128x128 array via tile_position column
    tiling, so decoding all 2048x512 weights costs ~7 us of PE time.
    Raw {0,1} bit planes stream over plain HWDGE DMA (fp8, 8 MiB).
  - ACT evicts n to fp16 (exact, <= 255); ACT computes sign(wsign) in
    place; one DVE multiply forms W_int = n * sign.
  - main matmul: psum[o=128, t=512] += W_int^T[i,o-blk] @ x^T[i,t-chunk]
    in fp16, fp32 PSUM accumulation; t-group-major with triple-buffered
    x streaming, 6 PSUM banks (2 reserved for decode).
  - eviction on ACT: y^T = Identity(psum * (scale/255) + bias_o), bf16
    output (host upcasts to f32; 2^-9 rounding ~10x under tolerance).
  - weights/bias/scalars are double-buffered across repeat iterations so
    iteration k+1's decode overlaps iteration k's matmuls: the in-NEFF
    repeat loop measures steady-state (pipelined) throughput.

Distribution over 8 NeuronCores: no collectives - each core writes its
own y^T shard, host reassembles.

Host-side work is layout/precision only: transposes, shard slicing, and
dtype conversion (bits {0.,1.} -> fp8 exact, x -> fp16, wsign -> bf16).
All of the module's arithmetic (bit-plane weighting and summation, sign,
scaling, matmul, bias) runs on the device. The decode matrix D is an
algorithm constant (like a transpose identity), not input data.
"""

import numpy as np

import concourse.mybir as mybir
import concourse.tile as tile
from concourse import bacc
from concourse import bass_utils

# ---- problem constants (hardcoded per contract) ----
B, S, IN, OUT, NB = 4, 2048, 2048, 2048, 8
T = B * S                      # 8192 tokens
P = 128                        # partitions
P_T, P_O = 2, 4                # token-parallel x out-feature-parallel grid
N_CORES = P_T * P_O
T_SH = T // P_T                # 4096
O_SH = OUT // P_O              # 512
KB = IN // P                   # 16 contraction blocks
OB = O_SH // P                 # 4 out blocks
TGW = 512                      # t-group width
NT = T_SH // TGW               # 8 t-groups per core

F32 = mybir.dt.float32
FP16 = mybir.dt.float16
BF16 = mybir.dt.bfloat16
FP8 = mybir.dt.float8e4
AL = mybir.AluOpType
IDENT = mybir.ActivationFunctionType.Identity

_CACHE = {}

import os as _os
_NO_DECODE = _os.environ.get("K_NO_DECODE", "0") == "1"  # memset W instead
_NO_MAIN = _os.environ.get("K_NO_MAIN", "0") == "1"      # decode only


def _pairs(ap):
    """Split the last (fast) axis of a [..., 2n] AP into even/odd views."""
    v = ap.rearrange("p (c two) -> p c two", two=2)
    return v[:, :, 0], v[:, :, 1]


def _decode_matrix():
    """D [128, 64] fp8: cols 0-31 hi-nibble decode, 32-63 lo-nibble.
    Row p = (i_sub, k) with i_sub = p//4, k = p%4 (MSB first)."""
    d = np.zeros((P, 64), dtype=np.float32)
    for p in range(P):
        i_sub, k = p // 4, p % 4
        d[p, i_sub] = 16.0 * (2.0 ** (3 - k))      # hi: 128,64,32,16
        d[p, 32 + i_sub] = 2.0 ** (3 - k)          # lo: 8,4,2,1
    return d


def _build_nc(repeats=1):
    nc = bacc.Bacc("TRN2", target_bir_lowering=False, debug=False,
                   num_devices=N_CORES)

    xgd = nc.dram_tensor("xg", [NT * P, KB * TGW], FP16,
                         kind="ExternalInput").ap()
    # raw bit planes, chunk order (kb, j, hi|lo), each chunk [128, 512]
    bits = nc.dram_tensor("bits", [P, KB * 4 * 2 * O_SH], FP8,
                          kind="ExternalInput").ap()
    wsd = nc.dram_tensor("ws", [P, KB * O_SH], BF16,
                         kind="ExternalInput").ap()
    dmat = nc.dram_tensor("dmat", [P, 64], FP8, kind="ExternalInput").ap()
    bb = nc.dram_tensor("bb", [P, OB * NB], F32, kind="ExternalInput").ap()
    bs = nc.dram_tensor("bs", [P, OB], F32, kind="ExternalInput").ap()
    scl = nc.dram_tensor("scl", [P, 1], F32, kind="ExternalInput").ap()
    bscl = nc.dram_tensor("bscl", [P, 1], F32, kind="ExternalInput").ap()
    y = nc.dram_tensor("y", [O_SH, T_SH], BF16, kind="ExternalOutput").ap()

    with tile.TileContext(nc) as tc:
      with tc.tile_pool(name="w", bufs=1) as wpool, \
           tc.tile_pool(name="xs", bufs=1) as xpool, \
           tc.tile_pool(name="dec", bufs=1) as dec, \
           tc.tile_pool(name="yb", bufs=1) as ypool, \
           tc.tile_pool(name="psum", bufs=1, space="PSUM") as psum_pool:

        dm = None
        if repeats:
            dm = dec.tile([P, 64], FP8, name="dmat_sb")
            nc.sync.dma_start(out=dm, in_=dmat)

        for rep in range(repeats):
            par = rep % 2   # parity suffix: double-buffer rep-crossing state

            # ---- scalars ----
            scl_sb = dec.tile([P, 1], F32, name=f"scl_sb{par}")
            nc.sync.dma_start(out=scl_sb, in_=scl)
            bscl_sb = dec.tile([P, 1], F32, name=f"bscl_sb{par}")
            nc.sync.dma_start(out=bscl_sb, in_=bscl)
            s255 = dec.tile([P, 1], F32, name=f"s255_{par}")
            nc.vector.tensor_scalar_mul(s255, scl_sb, 1.0 / 255.0)
            bs255 = dec.tile([P, 1], F32, name=f"bs255_{par}")
            nc.vector.tensor_scalar_mul(bs255, bscl_sb, 1.0 / 255.0)

            # ---- bias decode: bias_col [128, OB] (o on partitions) ----
            bb_sb = dec.tile([P, OB * NB], F32, name=f"bb_sb{par}")
            nc.sync.dma_start(out=bb_sb, in_=bb)
            bs_sb = dec.tile([P, OB], F32, name=f"bs_sb{par}")
            nc.sync.dma_start(out=bs_sb, in_=bs)
            e, o = _pairs(bb_sb)
            bl1 = dec.tile([P, OB * 4], F32, name=f"bl1_{par}")
            nc.vector.scalar_tensor_tensor(out=bl1, in0=e, scalar=2.0,
                                           in1=o, op0=AL.mult, op1=AL.add)
            e, o = _pairs(bl1)
            bl2 = dec.tile([P, OB * 2], F32, name=f"bl2_{par}")
            nc.vector.scalar_tensor_tensor(out=bl2, in0=e, scalar=4.0,
                                           in1=o, op0=AL.mult, op1=AL.add)
            e, o = _pairs(bl2)
            bl3 = dec.tile([P, OB], F32, name=f"bl3_{par}")
            nc.vector.scalar_tensor_tensor(out=bl3, in0=e, scalar=16.0,
                                           in1=o, op0=AL.mult, op1=AL.add)
            bsg = dec.tile([P, OB], F32, name=f"bsg{par}")
            nc.scalar.sign(bsg, bs_sb)
            bias_col = dec.tile([P, OB], F32, name=f"bias_col{par}")
            nc.vector.scalar_tensor_tensor(out=bias_col, in0=bl3,
                                           scalar=bs255, in1=bsg,
                                           op0=AL.mult, op1=AL.mult)

            # ---- weight sign (resident bf16, sign computed in place) ----
            wsg = dec.tile([P, KB * O_SH], BF16, name=f"wsg{par}")
            nc.sync.dma_start(out=wsg, in_=wsd)
            nc.scalar.sign(wsg, wsg)

            # ---- weight decode on the PE: n = D_hi.T@bits_hi + D_lo.T@
            #      bits_lo per 32-row block, 4 blocks col-tiled ----
            W = wpool.tile([P, KB * O_SH], FP16, name=f"W{par}")
            W3 = W.rearrange("p (kb o) -> p kb o", kb=KB)
            if _NO_DECODE:
                nc.vector.memset(W, 1.0)
            CHK = 2 * O_SH                       # hi+lo chunk pair cols
            for kb in range(KB if not _NO_DECODE else 0):
                bt = dec.tile([P, 4 * CHK], FP8, tag="bt", bufs=4)
                nc.sync.dma_start(
                    out=bt,
                    in_=bits[:, kb * 4 * CHK:(kb + 1) * 4 * CHK])
                bt3 = bt.rearrange("p (j h o) -> p j h o", j=4, h=2)
                ps = psum_pool.tile([P, O_SH], F32, tag="dec", bufs=2)
                for j in range(4):
                    nc.tensor.matmul(
                        ps[j * 32:(j + 1) * 32, :], dm[:, 0:32],
                        bt3[:, j, 0], start=True, stop=False,
                        tile_position=(0, j * 32), skip_group_check=True)
                    nc.tensor.matmul(
                        ps[j * 32:(j + 1) * 32, :], dm[:, 32:64],
                        bt3[:, j, 1], start=False, stop=True,
                        tile_position=(0, j * 32), skip_group_check=True)
                nc.scalar.activation(out=W3[:, kb], in_=ps, func=IDENT)
            # W_int = n * sign (exact fp16 integers, |n| <= 255)
            if not _NO_DECODE:
                nc.vector.tensor_tensor(out=W, in0=W, in1=wsg, op=AL.mult)

            # ---- main matmul: t-group-major, x triple-buffered ----
            def load_xg(g):
                xt = xpool.tile([P, KB * TGW], FP16, tag="xg",
                                name=f"xg{g}_{rep}", bufs=3)
                nc.sync.dma_start(out=xt, in_=xgd[g * P:(g + 1) * P, :])
                return xt

            xtiles = {g: load_xg(g) for g in range(min(3, NT))}
            for g in range(NT if not _NO_MAIN else 0):
                xg3 = xtiles[g].rearrange("p (kb t) -> p kb t", kb=KB)
                ybuf = ypool.tile([P, OB * TGW], BF16, tag="yb", bufs=2)
                yb3 = ybuf.rearrange("p (ob t) -> p ob t", ob=OB)
                for ob in range(OB):
                    ps = psum_pool.tile([P, TGW], F32, tag="mm", bufs=6)
                    for kb in range(KB):
                        nc.tensor.matmul(
                            ps,
                            W3[:, kb, ob * P:(ob + 1) * P],
                            xg3[:, kb],
                            start=(kb == 0),
                            stop=(kb == KB - 1),
                        )
                    # y^T tile = psum * (scale/255) + bias_o   (ACT)
                    nc.scalar.activation(
                        out=yb3[:, ob], in_=ps, func=IDENT,
                        bias=bias_col[:, ob:ob + 1], scale=s255)
                nc.sync.dma_start(
                    out=y.rearrange("(ob p) t -> p ob t", p=P)[
                        :, :, g * TGW:(g + 1) * TGW],
                    in_=yb3,
                )
                if g + 3 < NT:
                    xtiles[g + 3] = load_xg(g + 3)

    nc.compile()
    return nc


def _shard_inputs(x, bweight, wsign, scale, bbias, bsign, biasscale):
    fp8_np = mybir.dt.np(FP8)
    bf16_np = mybir.dt.np(BF16)

    x2 = np.asarray(x, dtype=np.float32).reshape(T, IN)
    bwf = np.asarray(bweight, dtype=np.float32)
    wsf = np.asarray(wsign, dtype=np.float32)
    bbias = np.asarray(bbias, dtype=np.float32)
    bsign = np.asarray(bsign, dtype=np.float32)

    scl_rep = np.full((P, 1), np.asarray(scale).reshape(-1)[0],
                      dtype=np.float32)
    bscl_rep = np.full((P, 1), np.asarray(biasscale).reshape(-1)[0],
                       dtype=np.float32)
    dmat_np = _decode_matrix().astype(fp8_np)

    o_maps = []
    for o_grp in range(P_O):
        osl = slice(o_grp * O_SH, (o_grp + 1) * O_SH)
        bw_sh = bwf[osl]                              # [O_SH, IN, NB]
        # bits chunks [128 = (32 i_sub x 4 k), O_SH], order (kb, j, hi|lo)
        bits_chunks = []
        for kb in range(KB):
            for j in range(4):
                i0 = kb * P + j * 32
                sub = bw_sh[:, i0:i0 + 32, :]         # [O_SH, 32, 8]
                for half in (0, 4):
                    bits_chunks.append(np.ascontiguousarray(
                        sub[:, :, half:half + 4].transpose(1, 2, 0)
                        .reshape(P, O_SH).astype(fp8_np)))
        # ws: [p, kb*O_SH + o] = sign weight for (i = kb*128+p, o)
        ws_np = np.ascontiguousarray(
            wsf[osl].T.reshape(KB, P, O_SH).transpose(1, 0, 2)
            .reshape(P, KB * O_SH)).astype(bf16_np)
        o_maps.append({
            "bits": np.concatenate(bits_chunks, axis=1),
            "ws": ws_np,
            "dmat": dmat_np,
            "bb": np.ascontiguousarray(
                bbias[osl].reshape(OB, P, NB).transpose(1, 0, 2)
                .reshape(P, OB * NB)),
            "bs": np.ascontiguousarray(bsign[osl].reshape(OB, P).T),
            "scl": scl_rep,
            "bscl": bscl_rep,
        })

    in_maps = [None] * N_CORES
    for t_grp in range(P_T):
        tsl = slice(t_grp * T_SH, (t_grp + 1) * T_SH)
        xs = x2[tsl]                                  # [T_SH, IN]
        xg_np = np.ascontiguousarray(
            xs.reshape(NT, TGW, KB, P).transpose(0, 3, 2, 1)
            .reshape(NT * P, KB * TGW).astype(np.float16))
        for o_grp in range(P_O):
            c = t_grp * P_O + o_grp
            in_maps[c] = dict(o_maps[o_grp], xg=xg_np)
    return in_maps


def kernel(x, bweight, wsign, scale, bbias, bsign, biasscale):
    if "nc" not in _CACHE:
        _CACHE["nc"] = _build_nc()
    nc = _CACHE["nc"]
    in_maps = _shard_inputs(x, bweight, wsign, scale, bbias, bsign, biasscale)
    res = bass_utils.run_bass_kernel_spmd(
        nc, in_maps, core_ids=list(range(N_CORES)))
    Y = np.empty((T, OUT), dtype=np.float32)
    for c in range(N_CORES):
        t_grp, o_grp = c // P_O, c % P_O
        Y[t_grp * T_SH:(t_grp + 1) * T_SH,
          o_grp * O_SH:(o_grp + 1) * O_SH] = \
            res.results[c]["y"].T.astype(np.float32)
    return Y.reshape(B, S, OUT)
